# revision 16
# baseline (speedup 1.0000x reference)
"""Trainium2 Bass kernel for the physics-informed MLP forecaster.

Model (per batch row of `history` [B, 24]):
  1. physics: 20-step delayed-feedback recurrence on the last history value
       T_new = (1-a)*T - b*T_delayed - g*T^3   (a,b = sigmoid(alpha/beta))
     with T_delayed from tau_int steps back (history first, then preds).
  2. x = [history(24) ; T_physics(20)] -> 3-layer tanh MLP (44->256^3)
     -> T_soft = c @ cor_w2 + cor_b2;  T_pred = T_physics + sigmoid(lm)*T_soft

Mapping (pure data parallel, 8 cores x 32768 rows; row = p*W + w on 128
partitions):
  * The physics recurrence is a serial DVE chain (~80 dependent ops, each
    pays its own-sem RAW wait), so it is CHUNKED over rows: a small first
    chunk (1/8 of rows) runs up front to unblock the MLP quickly, and each
    later chunk's recurrence+staging is interleaved into the DVE queue
    between the previous chunk's MLP tiles, hiding its latency entirely
    under the ACT-bound steady state.  Input DMAs are chunked to match so
    chunk 0 starts after ~0.5MB instead of 3.9MB of HBM traffic.
  * MLP is feature-major: per j-block the PE transposes comb16 [128,44]
    (fp16, 1 cyc/row) into PSUM; a DVE copy builds x^T [44,512] tiles.
    L1..L3 run fp16 matmuls (N=512); both M-halves share one 2-bank PSUM
    tile so tanh runs as ONE wide ACT op when biases are zero (they are
    structurally zero in setup_inputs; a per-half bias path handles the
    general case). L4 runs batch-major per j-block (lhsT = c^T block), so
    soft/pred staging is 2 batched DVE ops into the interleaved fp16
    [.,60] output tiles; 6 region DMAs stream them out (fp16 halves the
    store traffic and the exposed tail); host upcasts and splits 3 ways.
  * Steady state is ACT-bound (~213us of tanh on the Scalar engine, its
    throughput floor); PE is co-critical, and deeper tile interleaving
    that would hide the remaining ping-pong edges needs >8 PSUM banks,
    so the serial h->L2->f->L3->c chain per tile is structural.
  * This walrus build allows ONE sync-wait per instruction: engines
    "observe" parameter DMAs via tiny ops up front, provably-redundant
    same-engine WAW/WAR waits are pruned post-schedule, multi-wait tail
    drains are split into single-wait chains, and per-chunk/per-region
    tiles keep every instruction's dependency set to a single semaphore.
"""

import numpy as np

B = 262144
HIST = 24
FORE = 20
HID = 256
NCORES = 8
P = 128


def _build_nc(w, c1, bcoef, g, lam, tau_int, zero_bias=False):
    """Build the per-core Bass program. w = rows per partition (rows = 128*w)."""
    from contextlib import ExitStack

    import concourse.bass as bass
    import concourse.mybir as mybir
    import concourse.tile as tile

    f32 = mybir.dt.float32
    f16 = mybir.dt.float16
    AF = mybir.ActivationFunctionType
    ALU = mybir.AluOpType

    assert w % 4 == 0
    rows = P * w
    ntiles = w // 4  # 4 j-blocks (512 batch rows) per MLP tile

    # row chunks (per-partition widths); chunk 0 small to cut the serial
    # physics prefix, later chunks geometrically bigger since their
    # recurrence hides under the previous chunk's MLP tiles.
    if w == 256:
        cws = [32, 48, 80, 96]
    else:
        cws = [w]
    q0s = np.cumsum([0] + cws).tolist()
    nchunks = len(cws)
    chunk_t0 = [q0s[c] // 4 for c in range(nchunks)]
    chunk_t1 = [q0s[c + 1] // 4 for c in range(nchunks)]

    # output regions (tile ranges) for the chunked store DMAs; the last
    # region is small so the exposed post-loop DMA tail stays short.
    if ntiles == 64:
        out_regions = [(0, 12), (12, 24), (24, 44), (44, 58), (58, 62), (62, 64)]
    else:
        out_regions = [(0, ntiles)]

    nc = bass.Bass(trn_type="TRN2")

    WPK = HID + 2 * HID + 2 * HID + 2 * FORE + P  # w1 | w2 | w3 | w4 | ident16
    BPK = 6 + FORE + P  # b1|b2|b3 (2 cols each) | b4 broadcast | identity
    hist_d = nc.declare_dram_parameter("hist", [rows, HIST], f32, isOutput=False)
    htl_d = nc.declare_dram_parameter("htail", [rows, tau_int], f32, isOutput=False)
    wpk_d = nc.declare_dram_parameter("wpk", [P, WPK], f16, isOutput=False)
    bpk_d = nc.declare_dram_parameter("bpk", [P, BPK], f32, isOutput=False)
    out_d = nc.declare_dram_parameter("out60", [rows, 60], f16, isOutput=True)

    with ExitStack() as ctx:
        tc = ctx.enter_context(tile.TileContext(nc))
        const = ctx.enter_context(tc.tile_pool(name="const", bufs=1))
        xtp = ctx.enter_context(tc.tile_pool(name="xtp", bufs=3))
        hsb = ctx.enter_context(tc.tile_pool(name="hsb", bufs=3))
        pxp = ctx.enter_context(tc.tile_pool(name="pxp", bufs=1, space="PSUM"))
        php = ctx.enter_context(tc.tile_pool(name="php", bufs=1, space="PSUM"))
        spp = ctx.enter_context(tc.tile_pool(name="spp", bufs=1, space="PSUM"))

        # per-chunk input tiles (chunk granularity keeps each consumer's
        # dependency a single DMA-queue semaphore)
        hbs = [const.tile([P, cw * HIST], f32, name=f"hb{c}") for c, cw in enumerate(cws)]
        htls = [const.tile([P, cw * tau_int], f32, name=f"htl{c}") for c, cw in enumerate(cws)]
        pfs = [const.tile([P, cw * FORE], f32, name=f"pf{c}") for c, cw in enumerate(cws)]  # step-major
        hlasts = [const.tile([P, tau_int * cw], f32, name=f"hlast{c}") for c, cw in enumerate(cws)]
        cb16s = [const.tile([P, cw * (HIST + FORE)], f16, name=f"cb16_{c}") for c, cw in enumerate(cws)]
        # per-out-region interleaved output staging tiles
        st3s = [
            const.tile([P, (t1 - t0) * 4 * 60], f16, name=f"st3_{r}")
            for r, (t0, t1) in enumerate(out_regions)
        ]
        wpkt = const.tile([P, WPK], f16)
        bpkt = const.tile([P, BPK], f32)
        # physics scratch (sliced to chunk width; same-engine serial reuse)
        scr_u = const.tile([P, max(cws)], f32)
        scr_r = const.tile([P, max(cws)], f32)
        scr_s = const.tile([P, max(cws)], f32)

        # views into the packed parameter tiles
        NF = HIST + FORE  # 44 input features
        w1t = wpkt[0:NF, 0:HID]
        w2t = wpkt[:, HID : 3 * HID].rearrange("p (k m) -> p k m", k=2)
        w3t = wpkt[:, 3 * HID : 5 * HID].rearrange("p (k m) -> p k m", k=2)
        w4t = wpkt[:, 5 * HID : 5 * HID + 2 * FORE].rearrange(
            "p (k m) -> p k m", k=2
        )
        idt16 = wpkt[:, 5 * HID + 2 * FORE : 5 * HID + 2 * FORE + P]
        b1t = bpkt[:, 0:2]
        b2t = bpkt[:, 2:4]
        b3t = bpkt[:, 4:6]
        b4t = bpkt[:, 6 : 6 + FORE]
        idt = bpkt[:, 6 + FORE : 6 + FORE + P]

        # ---- input DMAs (chunked; queues assigned round-robin) ----
        hist_ap = hist_d[:].rearrange("(p q) c -> p (q c)", p=P)
        htl_ap = htl_d[:].rearrange("(p q) c -> p (q c)", p=P)
        nc.sync.dma_start(
            out=htls[0], in_=htl_ap[:, q0s[0] * tau_int : q0s[1] * tau_int]
        )
        nc.sync.dma_start(out=hbs[0], in_=hist_ap[:, q0s[0] * HIST : q0s[1] * HIST])
        nc.sync.dma_start(out=wpkt, in_=wpk_d[:])
        nc.sync.dma_start(out=bpkt, in_=bpk_d[:])
        for c in range(1, nchunks):
            nc.sync.dma_start(
                out=htls[c],
                in_=htl_ap[:, q0s[c] * tau_int : q0s[c + 1] * tau_int],
            )
            nc.sync.dma_start(
                out=hbs[c], in_=hist_ap[:, q0s[c] * HIST : q0s[c + 1] * HIST]
            )

        # "Observe" pass: with a 1-sync-wait budget per instruction, each
        # engine observes the parameter DMAs once up front via a tiny op, so
        # real matmuls/activations/DVE ops never need DMA waits of their own.
        obs = spp.tile([1, P], f32, tag="sp")
        nc.tensor.transpose(obs[0:1, 0:P], idt[:, 0:1], idt)  # bpk (ident)
        nc.tensor.transpose(obs[0:1, 0:P], wpkt[:, 0:2].bitcast(f32), idt)
        obs_a = const.tile([1, 1], f32)
        obs_v = const.tile([1, 1], f32)
        nc.scalar.copy(obs_a[0:1, 0:1], bpkt[0:1, 0:1])
        nc.scalar.activation(obs_a[0:1, 0:1], obs_a[0:1, 0:1], AF.Tanh)
        nc.vector.tensor_copy(obs_v[0:1, 0:1], bpkt[0:1, 0:1])

        out3 = out_d[:].rearrange("(p q) c -> p q c", p=P)
        st3ds = [s.rearrange("p (q c) -> p q c", c=60) for s in st3s]
        cb3ds = [s.rearrange("p (q c) -> p q c", c=HIST + FORE) for s in cb16s]

        def st3_region_of(q):
            """(region index, local q offset) for global j-block index q."""
            for r, (t0, t1) in enumerate(out_regions):
                if 4 * t0 <= q < 4 * t1:
                    return r, q - 4 * t0
            raise AssertionError(q)

        # ---- physics recurrence (DVE) thunks, per chunk ----
        # Strided DVE access runs at ~2 cycles/element, so the recurrence
        # state lives step-major per chunk: step s occupies the contiguous
        # run pf[:, s*cw:(s+1)*cw].  Each step is 4 serially-dependent DVE
        # ops; the chain latency only matters for chunk 0 (small), later
        # chunks drain between MLP tile bodies where the DVE has slack.
        def chunk_thunks(c):
            cw = cws[c]
            q0 = q0s[c]
            pf = pfs[c]
            hl = hlasts[c]
            htl = htls[c]
            thunks = []

            def t_hlast():
                src = bass.AP(
                    tensor=htl.tensor,
                    offset=htl.offset,
                    ap=[htl.ap[0], [1, tau_int], [tau_int, cw]],
                )
                nc.vector.tensor_copy(hl, src)

            thunks.append(t_hlast)
            for s in range(FORE):

                def t_step_u(s=s):
                    T = (
                        hl[:, (tau_int - 1) * cw : tau_int * cw]
                        if s == 0
                        else pf[:, (s - 1) * cw : s * cw]
                    )
                    nc.vector.tensor_tensor(
                        out=scr_u[:, 0:cw], in0=T, in1=T, op=ALU.mult
                    )

                def t_step_r(s=s):
                    T = (
                        hl[:, (tau_int - 1) * cw : tau_int * cw]
                        if s == 0
                        else pf[:, (s - 1) * cw : s * cw]
                    )
                    nc.vector.scalar_tensor_tensor(
                        out=scr_r[:, 0:cw],
                        in0=scr_u[:, 0:cw],
                        scalar=g,
                        in1=T,
                        op0=ALU.mult,
                        op1=ALU.mult,
                    )

                def t_step_t2(s=s):
                    Td = (
                        hl[:, s * cw : (s + 1) * cw]
                        if s < tau_int
                        else pf[:, (s - tau_int) * cw : (s - tau_int + 1) * cw]
                    )
                    nc.vector.scalar_tensor_tensor(
                        out=scr_s[:, 0:cw],
                        in0=Td,
                        scalar=bcoef,
                        in1=scr_r[:, 0:cw],
                        op0=ALU.mult,
                        op1=ALU.add,
                    )

                def t_step_tn(s=s):
                    T = (
                        hl[:, (tau_int - 1) * cw : tau_int * cw]
                        if s == 0
                        else pf[:, (s - 1) * cw : s * cw]
                    )
                    nc.vector.scalar_tensor_tensor(
                        out=pf[:, s * cw : (s + 1) * cw],
                        in0=T,
                        scalar=c1,
                        in1=scr_s[:, 0:cw],
                        op0=ALU.mult,
                        op1=ALU.subtract,
                    )

                thunks += [t_step_u, t_step_r, t_step_t2, t_step_tn]

            # staging: hist cast into the MLP input shadow, preds (fp32
            # exact) into the output tiles and (fp16) into the shadow.
            def stage_copy(out_ap, in_ap):
                nc.vector.tensor_copy(out_ap, in_ap)

            def t_hist_cast():
                stage_copy(
                    cb3ds[c][:, :, 0:HIST],
                    hbs[c].rearrange("p (q c) -> p q c", c=HIST),
                )

            def t_pred_cb(s0=0, s1=FORE):
                src = bass.AP(
                    tensor=pf.tensor,
                    offset=pf.offset + s0 * cw,
                    ap=[pf.ap[0], [1, cw], [cw, s1 - s0]],
                )
                stage_copy(cb3ds[c][:, :, HIST + s0 : HIST + s1], src)

            # first step-half stages mid-chain (hides under steps 10..19)
            thunks.insert(1 + 4 * (FORE // 2), lambda: t_pred_cb(0, FORE // 2))
            thunks += [t_hist_cast, lambda: t_pred_cb(FORE // 2, FORE)]

            # st3 physics staging, split on out-region boundaries
            qa = q0
            while qa < q0 + cw:
                r, qloc = st3_region_of(qa)
                qb = min(q0 + cw, 4 * out_regions[r][1])
                n = qb - qa

                def t_pred_st3(r=r, qloc=qloc, qa=qa, n=n):
                    src = bass.AP(
                        tensor=pf.tensor,
                        offset=pf.offset + (qa - q0),
                        ap=[pf.ap[0], [1, n], [cw, FORE]],
                    )
                    nc.vector.tensor_copy(
                        st3ds[r][:, qloc : qloc + n, 40:60], src
                    )

                thunks.append(t_pred_st3)
                qa = qb
            return thunks

        # chunk 0 runs up front (short serial prefix), later chunks drain
        # between tile bodies below.
        for th in chunk_thunks(0):
            th()

        # ---- MLP over tiles of 4 j-blocks (512 batch rows) ----
        NB = 4 * P  # moving free dim
        pending = []
        per_tile = 0
        for t in range(ntiles):
            ci = next(
                c for c in range(nchunks) if chunk_t0[c] <= t < chunk_t1[c]
            )
            if t == chunk_t0[ci] and ci + 1 < nchunks:
                pending = chunk_thunks(ci + 1)
                # skip the window's first tile (pipeline refill), finish two
                # tiles before the window ends so the next chunk's first
                # transposes never wait on staging
                nt = max(1, chunk_t1[ci] - chunk_t0[ci] - 3)
                per_tile = -(-len(pending) // nt)

            q0 = q0s[ci]
            cb16 = cb16s[ci]
            px = pxp.tile([64, NB], f16, tag="px")
            for jl in range(4):
                jloc = 4 * t + jl - q0
                # x^T block: [128, 44] f16 -> [44, 128] f16 in PSUM
                nc.tensor.transpose(
                    px[0:NF, jl * P : (jl + 1) * P],
                    cb16[:, jloc * NF : (jloc + 1) * NF],
                    idt16,
                )
            xt = xtp.tile([64, NB], f16, tag="xt")
            nc.vector.tensor_copy(xt[0:NF, :], px[0:NF, :])
            # drain the next chunk's physics/staging into the DVE queue
            # here (after the ACT-critical xt copy, before soft/pred)
            if t != chunk_t0[ci]:
                for _ in range(per_tile):
                    if pending:
                        pending.pop(0)()
            # PE observe of the DVE clock (covers the xt copy and all older
            # DVE work, incl. the previous tile's soft/pred staging) so the
            # matmuls below need no DVE sync-wait of their own.
            nc.tensor.transpose(
                px[0:1, 0:2].bitcast(f32), xt[0:1, 0:2].bitcast(f32),
                idt[0:1, 0:1],
            )

            # Each layer: both M-halves matmul into one 2-bank PSUM tile;
            # with zero biases the tanh runs as ONE wide ACT op (halves the
            # ~352-cycle per-op ACT overhead), else per-half with bias.
            def layer(tag, lhsT_of, rhs_of, bias):
                pp = php.tile([P, 2 * NB], f32, tag=tag)
                for m in range(2):
                    for k, (lhsT, sstop) in enumerate(lhsT_of(m)):
                        nc.tensor.matmul(
                            pp[:, m * NB : (m + 1) * NB],
                            lhsT,
                            rhs_of(k),
                            start=(k == 0),
                            stop=sstop,
                        )
                ot = hsb.tile([P, 2 * NB], f16, tag=tag + "s")
                if zero_bias:
                    nc.scalar.activation(ot, pp, AF.Tanh)
                else:
                    for m in range(2):
                        nc.scalar.activation(
                            ot[:, m * NB : (m + 1) * NB],
                            pp[:, m * NB : (m + 1) * NB],
                            AF.Tanh,
                            bias=bias[:, m : m + 1],
                        )
                return ot

            htb = layer(
                "h",
                lambda m: [(w1t[:, m * P : (m + 1) * P], True)],
                lambda k: xt[0:NF, :],
                b1t,
            )
            hts = [htb[:, 0:NB], htb[:, NB : 2 * NB]]
            ftb = layer(
                "f",
                lambda m: [
                    (w2t[:, 0, m * P : (m + 1) * P], False),
                    (w2t[:, 1, m * P : (m + 1) * P], True),
                ],
                lambda k: hts[k],
                b2t,
            )
            fts = [ftb[:, 0:NB], ftb[:, NB : 2 * NB]]
            ctb = layer(
                "c",
                lambda m: [
                    (w3t[:, 0, m * P : (m + 1) * P], False),
                    (w3t[:, 1, m * P : (m + 1) * P], True),
                ],
                lambda k: fts[k],
                b3t,
            )
            cts = [ctb[:, 0:NB], ctb[:, NB : 2 * NB]]

            # L4 batch-major per j-block: T_soft[128,20] = (c^T block).T @ w4.
            # All 4 j-blocks share one PSUM tile (one bank) so the soft/pred
            # staging below is 2 batched DVE ops per tile.
            sp = spp.tile([P, 4 * FORE], f32, tag="sp")
            for jl in range(4):
                for k in range(2):
                    nc.tensor.matmul(
                        sp[:, jl * FORE : (jl + 1) * FORE],
                        cts[k][:, jl * P : (jl + 1) * P],
                        w4t[:, k, :],
                        start=(k == 0),
                        stop=(k == 1),
                    )
            sp3 = sp.rearrange("p (q c) -> p q c", c=FORE)
            b4b = b4t.unsqueeze(1).broadcast_to((P, 4, FORE))
            r, qloc = st3_region_of(4 * t)
            soft = st3ds[r][:, qloc : qloc + 4, 0:FORE]
            pred = st3ds[r][:, qloc : qloc + 4, FORE : 2 * FORE]
            phys = st3ds[r][:, qloc : qloc + 4, 2 * FORE : 3 * FORE]
            nc.vector.tensor_tensor(out=soft, in0=sp3, in1=b4b, op=ALU.add)
            nc.vector.scalar_tensor_tensor(
                out=pred, in0=soft, scalar=lam, in1=phys, op0=ALU.mult, op1=ALU.add
            )

            # chunked output DMAs (each region first-on-its-queue)
            for r2, (t0r, t1r) in enumerate(out_regions):
                if t + 1 == t1r:
                    nc.sync.dma_start(
                        out=out3[:, 4 * t0r : 4 * t1r, :], in_=st3s[r2]
                    )

    _prune_redundant_waits(nc)
    _split_fat_drains(nc)
    return nc


def _split_fat_drains(nc):
    """Split multi-wait drains into chains of single-wait drains.

    Every instruction struct in this walrus build accepts one sync wait;
    the Tile kernel-tail drain gathers all procs on one instruction. A
    sequence of drains on the same in-order queue is semantically
    identical.
    """
    import concourse.mybir as mybir

    fn = nc.m.functions[0]
    for bb in fn.blocks:
        il = bb.instructions
        idx = 0
        while idx < len(il):
            inst = il[idx]
            si = inst.sync_info
            if (
                isinstance(inst, mybir.InstDrain)
                and si
                and si.on_wait
                and len(si.on_wait) > 1
            ):
                waits = list(si.on_wait)
                for j, wt in enumerate(waits[:-1]):
                    d = mybir.InstDrain(name=f"{inst.name}-w{j}", ins=[], outs=[])
                    d.engine = inst.engine
                    d.sync_info = mybir.SyncInfo(on_wait=[wt], on_update=[])
                    try:
                        nc.register_instruction(d, overwrite=True)
                    except Exception:
                        pass
                    il.insert(idx, d)
                    idx += 1
                si.on_wait = [waits[-1]]
            idx += 1


def _prune_redundant_waits(nc):
    """Drop statically-redundant same-proc semaphore waits.

    Tile's slot-rotation deps stamp the released tile's full accessor clock
    onto the next user, including waits on the instruction's *own* in-order
    proc (engine completion sems / its own DMA queue's sem). Those are
    satisfied by program order, but this walrus build only allows ONE sync
    wait per instruction, so the redundant ones must go. A wait is pruned
    only when every increment of its semaphore comes from earlier
    instructions of the same proc stream (verified by cumulative count).
    CoreSim (race detector + deadlock check) validates the pruned program.
    """
    # Same-engine waits are needed only for same-engine RAW hazards (a read
    # racing an earlier posted write from the same engine). In this program:
    #   * PE reads only SBUF and writes only PSUM  -> no PE-self RAW ever
    #   * ACT reads only PSUM/bias and writes SBUF tiles nothing on ACT
    #     reads back                               -> no ACT-self RAW ever
    #   * DVE reads its own writes constantly (physics recurrence, pred
    #     reading soft), EXCEPT the px->xt copies whose only input is
    #     PE-written PSUM                          -> prune only on xt copies
    # WAW/WAR same-engine edges are enforced by in-order execution and the
    # engine's FIFO write path. DMA queue-self waits order transfers on the
    # same FIFO ring, which processes descriptors serially anyway.
    eng_sem_prefix = {
        "EngineType.PE": "PE_",
        "EngineType.DVE": "DVE_",
        "EngineType.Activation": "Activation_",
        "EngineType.SP": "SP_",
        "EngineType.Pool": "Pool_",
    }
    fn = nc.m.functions[0]
    insts = [i for bb in fn.blocks for i in bb.instructions]
    updaters = {}
    for inst in insts:
        si = inst.sync_info
        if si and si.on_update:
            for u in si.on_update:
                nm = getattr(u, "ant_name", None)
                if nm:
                    updaters.setdefault(nm, set()).add(str(inst.engine))
    cum = {}
    pruned = 0
    for inst in insts:
        si = inst.sync_info
        eng = str(inst.engine)
        tname = type(inst).__name__
        try:
            out_ref = inst.outs[0].memref
        except Exception:
            out_ref = ""
        if si and si.on_wait:
            own_updates = set()
            for u in si.on_update or []:
                nm = getattr(u, "ant_name", None)
                if nm:
                    own_updates.add(nm)
            keep = []
            for wt in si.on_wait:
                nm = wt.ant_name
                prunable = False
                if nm and nm.startswith(eng_sem_prefix.get(eng, "\x00")) and (
                    updaters.get(nm, set()) <= {eng}
                ):
                    if eng == "EngineType.PE":
                        prunable = True  # PE never reads PE-written data
                    elif eng == "EngineType.Activation":
                        prunable = True  # ACT never reads ACT-written data
                    elif eng == "EngineType.DVE" and out_ref.startswith("xt_"):
                        prunable = True  # xt copy reads only PE-written PSUM
                if nm and nm.startswith("DMAHW") and tname == "InstDMACopy":
                    # Ring-FIFO ordering wait vs an earlier DMA that shares
                    # the ring. Every DMA pair in this program touches
                    # disjoint memory (chunked inputs / region outputs), and
                    # a ring processes its descriptors serially anyway, so
                    # the wait carries no data hazard.
                    prunable = True
                if prunable and wt.wait_value <= cum.get(nm, 0):
                    pruned += 1
                    continue
                keep.append(wt)
            if len(keep) != len(si.on_wait):
                si.on_wait = keep
        if si and si.on_update:
            for u in si.on_update:
                nm = getattr(u, "ant_name", None)
                if nm:
                    cum[nm] = cum.get(nm, 0) + getattr(u, "update_value", 1)
    return pruned


def _prep_weights(enc_w1, enc_b1, enc_w2, enc_b2, cor_w1, cor_b1, cor_w2, cor_b2):
    f32, f16 = np.float32, np.float16
    WPK = HID + 2 * HID + 2 * HID + 2 * FORE + P
    wpk = np.zeros((P, WPK), f16)
    wpk[:, 5 * HID + 2 * FORE : 5 * HID + 2 * FORE + P] = np.eye(P, dtype=f16)
    wpk[0 : HIST + FORE, 0:HID] = enc_w1.astype(f16)
    wpk[:, HID : 3 * HID] = (
        enc_w2.reshape(2, P, HID).transpose(1, 0, 2).reshape(P, 2 * HID).astype(f16)
    )
    wpk[:, 3 * HID : 5 * HID] = (
        cor_w1.reshape(2, P, HID).transpose(1, 0, 2).reshape(P, 2 * HID).astype(f16)
    )
    wpk[:, 5 * HID : 5 * HID + 2 * FORE] = (
        cor_w2.reshape(2, P, FORE).transpose(1, 0, 2).reshape(P, 2 * FORE).astype(f16)
    )
    BPK = 6 + FORE + P
    bpk = np.zeros((P, BPK), f32)
    bpk[:, 0:2] = enc_b1.reshape(2, P).T
    bpk[:, 2:4] = enc_b2.reshape(2, P).T
    bpk[:, 4:6] = cor_b1.reshape(2, P).T
    bpk[:, 6 : 6 + FORE] = np.broadcast_to(cor_b2.reshape(1, FORE), (P, FORE))
    bpk[:, 6 + FORE : 6 + FORE + P] = np.eye(P, dtype=f32)
    return dict(wpk=wpk, bpk=bpk)


LAST_RESULT = None  # BassKernelResults of the most recent kernel() call


def kernel(history, enc_w1, enc_b1, enc_w2, enc_b2, cor_w1, cor_b1, cor_w2, cor_b2,
           alpha, beta, gamma, tau, lambda_mix):
    from concourse.bass_utils import run_bass_kernel_spmd

    global LAST_RESULT

    history = np.asarray(history, np.float32)
    assert history.shape == (B, HIST)

    def sig(x):
        return float(1.0 / (1.0 + np.exp(-np.float64(x))))

    a = sig(alpha)
    bcoef = sig(beta)
    g = float(abs(np.float64(gamma)))
    lam = sig(lambda_mix)
    c1 = 1.0 - a
    tau_int = int(np.clip(float(tau), 1.0, 18.0))

    zb = not (
        np.any(np.asarray(enc_b1)) or np.any(np.asarray(enc_b2))
        or np.any(np.asarray(cor_b1))
    )
    w = B // NCORES // P  # rows per partition per core
    nc = _build_nc(w, c1, bcoef, g, lam, tau_int, zero_bias=zb)

    shared = _prep_weights(
        np.asarray(enc_w1, np.float32), np.asarray(enc_b1, np.float32),
        np.asarray(enc_w2, np.float32), np.asarray(enc_b2, np.float32),
        np.asarray(cor_w1, np.float32), np.asarray(cor_b1, np.float32),
        np.asarray(cor_w2, np.float32), np.asarray(cor_b2, np.float32),
    )
    rows = B // NCORES
    htail_full = np.ascontiguousarray(history[:, HIST - tau_int :])
    in_maps = [
        {
            "hist": np.ascontiguousarray(history[i * rows : (i + 1) * rows]),
            "htail": htail_full[i * rows : (i + 1) * rows],
            **shared,
        }
        for i in range(NCORES)
    ]

    res = run_bass_kernel_spmd(nc, in_maps, core_ids=list(range(NCORES)))
    LAST_RESULT = res

    preds, physs, softs = [], [], []
    for i in range(NCORES):
        o = np.asarray(res.results[i]["out60"], np.float32).reshape(rows, 60)
        softs.append(o[:, 0:FORE])
        preds.append(o[:, FORE : 2 * FORE])
        physs.append(o[:, 2 * FORE : 3 * FORE])
    T_soft = np.concatenate(softs, 0)
    T_pred = np.concatenate(preds, 0)
    T_physics = np.concatenate(physs, 0)
    return (T_pred, T_physics, T_soft)


# revision 17
# speedup vs baseline: 1.0043x; 1.0043x over previous
"""Trainium2 Bass kernel for the physics-informed MLP forecaster.

Model (per batch row of `history` [B, 24]):
  1. physics: 20-step delayed-feedback recurrence on the last history value
       T_new = (1-a)*T - b*T_delayed - g*T^3   (a,b = sigmoid(alpha/beta))
     with T_delayed from tau_int steps back (history first, then preds).
  2. x = [history(24) ; T_physics(20)] -> 3-layer tanh MLP (44->256^3)
     -> T_soft = c @ cor_w2 + cor_b2;  T_pred = T_physics + sigmoid(lm)*T_soft

Mapping (pure data parallel, 8 cores x 32768 rows; row = p*W + w on 128
partitions):
  * The physics recurrence is a serial DVE chain (~80 dependent ops, each
    pays its own-sem RAW wait), so it is CHUNKED over rows: a small first
    chunk (1/8 of rows) runs up front to unblock the MLP quickly, and each
    later chunk's recurrence+staging is interleaved into the DVE queue
    between the previous chunk's MLP tiles, hiding its latency entirely
    under the ACT-bound steady state.  Input DMAs are chunked to match so
    chunk 0 starts after ~0.5MB instead of 3.9MB of HBM traffic.
  * MLP is feature-major: per j-block the PE transposes comb16 [128,44]
    (fp16, 1 cyc/row) into PSUM; a DVE copy builds x^T [44,512] tiles.
    L1..L3 run fp16 matmuls (N=512); both M-halves share one 2-bank PSUM
    tile so tanh runs as ONE wide ACT op when biases are zero (they are
    structurally zero in setup_inputs; a per-half bias path handles the
    general case). L4 runs batch-major per j-block (lhsT = c^T block), so
    soft/pred staging is 2 batched DVE ops into the interleaved fp16
    [.,60] output tiles; 6 region DMAs stream them out (fp16 halves the
    store traffic and the exposed tail); host upcasts and splits 3 ways.
  * Steady state is ACT-bound (~213us of tanh on the Scalar engine, its
    throughput floor); PE is co-critical, and deeper tile interleaving
    that would hide the remaining ping-pong edges needs >8 PSUM banks,
    so the serial h->L2->f->L3->c chain per tile is structural.
  * This walrus build allows ONE sync-wait per instruction: engines
    "observe" parameter DMAs via tiny ops up front, provably-redundant
    same-engine WAW/WAR waits are pruned post-schedule, multi-wait tail
    drains are split into single-wait chains, and per-chunk/per-region
    tiles keep every instruction's dependency set to a single semaphore.
"""

import numpy as np

B = 262144
HIST = 24
FORE = 20
HID = 256
NCORES = 8
P = 128


def _build_nc(w, c1, bcoef, g, lam, tau_int, zero_bias=False):
    """Build the per-core Bass program. w = rows per partition (rows = 128*w)."""
    from contextlib import ExitStack

    import concourse.bass as bass
    import concourse.mybir as mybir
    import concourse.tile as tile

    f32 = mybir.dt.float32
    f16 = mybir.dt.float16
    AF = mybir.ActivationFunctionType
    ALU = mybir.AluOpType

    assert w % 4 == 0
    rows = P * w
    ntiles = w // 4  # 4 j-blocks (512 batch rows) per MLP tile

    # row chunks (per-partition widths); chunk 0 small to cut the serial
    # physics prefix, later chunks geometrically bigger since their
    # recurrence hides under the previous chunk's MLP tiles.
    if w == 256:
        cws = [32, 48, 80, 96]
    else:
        cws = [w]
    q0s = np.cumsum([0] + cws).tolist()
    nchunks = len(cws)
    chunk_t0 = [q0s[c] // 4 for c in range(nchunks)]
    chunk_t1 = [q0s[c + 1] // 4 for c in range(nchunks)]

    # output regions (tile ranges) for the chunked store DMAs; the last
    # region is small so the exposed post-loop DMA tail stays short.
    if ntiles == 64:
        out_regions = [(0, 12), (12, 24), (24, 44), (44, 58), (58, 62), (62, 64)]
    else:
        out_regions = [(0, ntiles)]

    nc = bass.Bass(trn_type="TRN2")

    WPK = HID + 2 * HID + 2 * HID + 2 * FORE + P  # w1 | w2 | w3 | w4 | ident16
    BPK = 6 + FORE + P  # b1|b2|b3 (2 cols each) | b4 broadcast | identity
    hist_d = nc.declare_dram_parameter("hist", [rows, HIST], f32, isOutput=False)
    htl_d = nc.declare_dram_parameter("htail", [rows, tau_int], f32, isOutput=False)
    wpk_d = nc.declare_dram_parameter("wpk", [P, WPK], f16, isOutput=False)
    bpk_d = nc.declare_dram_parameter("bpk", [P, BPK], f32, isOutput=False)
    out_d = nc.declare_dram_parameter("out60", [rows, 60], f16, isOutput=True)

    with ExitStack() as ctx:
        tc = ctx.enter_context(tile.TileContext(nc))
        const = ctx.enter_context(tc.tile_pool(name="const", bufs=1))
        xtp = ctx.enter_context(tc.tile_pool(name="xtp", bufs=3))
        hsb = ctx.enter_context(tc.tile_pool(name="hsb", bufs=3))
        pxp = ctx.enter_context(tc.tile_pool(name="pxp", bufs=1, space="PSUM"))
        php = ctx.enter_context(tc.tile_pool(name="php", bufs=1, space="PSUM"))
        spp = ctx.enter_context(tc.tile_pool(name="spp", bufs=1, space="PSUM"))

        # per-chunk input tiles (chunk granularity keeps each consumer's
        # dependency a single DMA-queue semaphore)
        hbs = [const.tile([P, cw * HIST], f32, name=f"hb{c}") for c, cw in enumerate(cws)]
        htls = [const.tile([P, cw * tau_int], f32, name=f"htl{c}") for c, cw in enumerate(cws)]
        pfs = [const.tile([P, cw * FORE], f32, name=f"pf{c}") for c, cw in enumerate(cws)]  # step-major
        hlasts = [const.tile([P, tau_int * cw], f32, name=f"hlast{c}") for c, cw in enumerate(cws)]
        cb16s = [const.tile([P, cw * (HIST + FORE)], f16, name=f"cb16_{c}") for c, cw in enumerate(cws)]
        # per-out-region interleaved output staging tiles
        st3s = [
            const.tile([P, (t1 - t0) * 4 * 60], f16, name=f"st3_{r}")
            for r, (t0, t1) in enumerate(out_regions)
        ]
        wpkt = const.tile([P, WPK], f16)
        bpkt = const.tile([P, BPK], f32)
        # physics scratch (sliced to chunk width; same-engine serial reuse)
        scr_u = const.tile([P, max(cws)], f32)
        scr_r = const.tile([P, max(cws)], f32)
        scr_s = const.tile([P, max(cws)], f32)

        # views into the packed parameter tiles
        NF = HIST + FORE  # 44 input features
        w1t = wpkt[0:NF, 0:HID]
        w2t = wpkt[:, HID : 3 * HID].rearrange("p (k m) -> p k m", k=2)
        w3t = wpkt[:, 3 * HID : 5 * HID].rearrange("p (k m) -> p k m", k=2)
        w4t = wpkt[:, 5 * HID : 5 * HID + 2 * FORE].rearrange(
            "p (k m) -> p k m", k=2
        )
        idt16 = wpkt[:, 5 * HID + 2 * FORE : 5 * HID + 2 * FORE + P]
        b1t = bpkt[:, 0:2]
        b2t = bpkt[:, 2:4]
        b3t = bpkt[:, 4:6]
        b4t = bpkt[:, 6 : 6 + FORE]
        idt = bpkt[:, 6 + FORE : 6 + FORE + P]

        # ---- input DMAs (chunked; queues assigned round-robin) ----
        hist_ap = hist_d[:].rearrange("(p q) c -> p (q c)", p=P)
        htl_ap = htl_d[:].rearrange("(p q) c -> p (q c)", p=P)
        nc.sync.dma_start(
            out=htls[0], in_=htl_ap[:, q0s[0] * tau_int : q0s[1] * tau_int]
        )
        nc.sync.dma_start(out=hbs[0], in_=hist_ap[:, q0s[0] * HIST : q0s[1] * HIST])
        nc.sync.dma_start(out=wpkt, in_=wpk_d[:])
        nc.sync.dma_start(out=bpkt, in_=bpk_d[:])
        for c in range(1, nchunks):
            nc.sync.dma_start(
                out=htls[c],
                in_=htl_ap[:, q0s[c] * tau_int : q0s[c + 1] * tau_int],
            )
            nc.sync.dma_start(
                out=hbs[c], in_=hist_ap[:, q0s[c] * HIST : q0s[c + 1] * HIST]
            )

        # "Observe" pass: with a 1-sync-wait budget per instruction, each
        # engine observes the parameter DMAs once up front via a tiny op, so
        # real matmuls/activations/DVE ops never need DMA waits of their own.
        obs = spp.tile([1, P], f32, tag="sp")
        nc.tensor.transpose(obs[0:1, 0:P], idt[:, 0:1], idt)  # bpk (ident)
        nc.tensor.transpose(obs[0:1, 0:P], wpkt[:, 0:2].bitcast(f32), idt)
        obs_a = const.tile([1, 1], f32)
        obs_v = const.tile([1, 1], f32)
        nc.scalar.copy(obs_a[0:1, 0:1], bpkt[0:1, 0:1])
        nc.vector.tensor_copy(obs_v[0:1, 0:1], bpkt[0:1, 0:1])

        out3 = out_d[:].rearrange("(p q) c -> p q c", p=P)
        st3ds = [s.rearrange("p (q c) -> p q c", c=60) for s in st3s]
        cb3ds = [s.rearrange("p (q c) -> p q c", c=HIST + FORE) for s in cb16s]

        def st3_region_of(q):
            """(region index, local q offset) for global j-block index q."""
            for r, (t0, t1) in enumerate(out_regions):
                if 4 * t0 <= q < 4 * t1:
                    return r, q - 4 * t0
            raise AssertionError(q)

        # ---- physics recurrence (DVE) thunks, per chunk ----
        # Strided DVE access runs at ~2 cycles/element, so the recurrence
        # state lives step-major per chunk: step s occupies the contiguous
        # run pf[:, s*cw:(s+1)*cw].  Each step is 4 serially-dependent DVE
        # ops; the chain latency only matters for chunk 0 (small), later
        # chunks drain between MLP tile bodies where the DVE has slack.
        def chunk_thunks(c):
            cw = cws[c]
            q0 = q0s[c]
            pf = pfs[c]
            hl = hlasts[c]
            htl = htls[c]
            thunks = []

            def t_hlast():
                src = bass.AP(
                    tensor=htl.tensor,
                    offset=htl.offset,
                    ap=[htl.ap[0], [1, tau_int], [tau_int, cw]],
                )
                nc.vector.tensor_copy(hl, src)

            thunks.append(t_hlast)
            for s in range(FORE):

                def t_step_u(s=s):
                    T = (
                        hl[:, (tau_int - 1) * cw : tau_int * cw]
                        if s == 0
                        else pf[:, (s - 1) * cw : s * cw]
                    )
                    nc.vector.tensor_tensor(
                        out=scr_u[:, 0:cw], in0=T, in1=T, op=ALU.mult
                    )

                def t_step_r(s=s):
                    T = (
                        hl[:, (tau_int - 1) * cw : tau_int * cw]
                        if s == 0
                        else pf[:, (s - 1) * cw : s * cw]
                    )
                    nc.vector.scalar_tensor_tensor(
                        out=scr_r[:, 0:cw],
                        in0=scr_u[:, 0:cw],
                        scalar=g,
                        in1=T,
                        op0=ALU.mult,
                        op1=ALU.mult,
                    )

                def t_step_t2(s=s):
                    Td = (
                        hl[:, s * cw : (s + 1) * cw]
                        if s < tau_int
                        else pf[:, (s - tau_int) * cw : (s - tau_int + 1) * cw]
                    )
                    nc.vector.scalar_tensor_tensor(
                        out=scr_s[:, 0:cw],
                        in0=Td,
                        scalar=bcoef,
                        in1=scr_r[:, 0:cw],
                        op0=ALU.mult,
                        op1=ALU.add,
                    )

                def t_step_tn(s=s):
                    T = (
                        hl[:, (tau_int - 1) * cw : tau_int * cw]
                        if s == 0
                        else pf[:, (s - 1) * cw : s * cw]
                    )
                    nc.vector.scalar_tensor_tensor(
                        out=pf[:, s * cw : (s + 1) * cw],
                        in0=T,
                        scalar=c1,
                        in1=scr_s[:, 0:cw],
                        op0=ALU.mult,
                        op1=ALU.subtract,
                    )

                thunks += [t_step_u, t_step_r, t_step_t2, t_step_tn]

            # staging: hist cast into the MLP input shadow, preds (fp32
            # exact) into the output tiles and (fp16) into the shadow.
            def stage_copy(out_ap, in_ap):
                nc.vector.tensor_copy(out_ap, in_ap)

            def t_hist_cast():
                stage_copy(
                    cb3ds[c][:, :, 0:HIST],
                    hbs[c].rearrange("p (q c) -> p q c", c=HIST),
                )

            def t_pred_cb(s0=0, s1=FORE):
                src = bass.AP(
                    tensor=pf.tensor,
                    offset=pf.offset + s0 * cw,
                    ap=[pf.ap[0], [1, cw], [cw, s1 - s0]],
                )
                stage_copy(cb3ds[c][:, :, HIST + s0 : HIST + s1], src)

            # first step-half stages mid-chain (hides under steps 10..19)
            thunks.insert(1 + 4 * (FORE // 2), lambda: t_pred_cb(0, FORE // 2))
            thunks += [t_hist_cast, lambda: t_pred_cb(FORE // 2, FORE)]

            # st3 physics staging, split on out-region boundaries
            qa = q0
            while qa < q0 + cw:
                r, qloc = st3_region_of(qa)
                qb = min(q0 + cw, 4 * out_regions[r][1])
                n = qb - qa

                def t_pred_st3(r=r, qloc=qloc, qa=qa, n=n):
                    src = bass.AP(
                        tensor=pf.tensor,
                        offset=pf.offset + (qa - q0),
                        ap=[pf.ap[0], [1, n], [cw, FORE]],
                    )
                    nc.vector.tensor_copy(
                        st3ds[r][:, qloc : qloc + n, 40:60], src
                    )

                thunks.append(t_pred_st3)
                qa = qb
            return thunks

        # chunk 0 runs up front (short serial prefix), later chunks drain
        # between tile bodies below.
        for th in chunk_thunks(0):
            th()

        # ---- MLP over tiles of 4 j-blocks (512 batch rows) ----
        NB = 4 * P  # moving free dim
        pending = []
        per_tile = 0
        for t in range(ntiles):
            ci = next(
                c for c in range(nchunks) if chunk_t0[c] <= t < chunk_t1[c]
            )
            if t == chunk_t0[ci] and ci + 1 < nchunks:
                pending = chunk_thunks(ci + 1)
                # skip the window's first tile (pipeline refill), finish two
                # tiles before the window ends so the next chunk's first
                # transposes never wait on staging
                nt = max(1, chunk_t1[ci] - chunk_t0[ci] - 3)
                per_tile = -(-len(pending) // nt)

            q0 = q0s[ci]
            cb16 = cb16s[ci]
            px = pxp.tile([64, NB], f16, tag="px")
            for jl in range(4):
                jloc = 4 * t + jl - q0
                # x^T block: [128, 44] f16 -> [44, 128] f16 in PSUM
                nc.tensor.transpose(
                    px[0:NF, jl * P : (jl + 1) * P],
                    cb16[:, jloc * NF : (jloc + 1) * NF],
                    idt16,
                )
            xt = xtp.tile([64, NB], f16, tag="xt")
            nc.vector.tensor_copy(xt[0:NF, :], px[0:NF, :])
            # drain the next chunk's physics/staging into the DVE queue
            # here (after the ACT-critical xt copy, before soft/pred)
            if t != chunk_t0[ci]:
                for _ in range(per_tile):
                    if pending:
                        pending.pop(0)()
            # PE observe of the DVE clock (covers the xt copy and all older
            # DVE work, incl. the previous tile's soft/pred staging) so the
            # matmuls below need no DVE sync-wait of their own.
            nc.tensor.transpose(
                px[0:1, 0:2].bitcast(f32), xt[0:1, 0:2].bitcast(f32),
                idt[0:1, 0:1],
            )

            # Each layer: both M-halves matmul into one 2-bank PSUM tile;
            # with zero biases the tanh runs as ONE wide ACT op (halves the
            # ~352-cycle per-op ACT overhead), else per-half with bias.
            def layer(tag, lhsT_of, rhs_of, bias):
                pp = php.tile([P, 2 * NB], f32, tag=tag)
                for m in range(2):
                    for k, (lhsT, sstop) in enumerate(lhsT_of(m)):
                        nc.tensor.matmul(
                            pp[:, m * NB : (m + 1) * NB],
                            lhsT,
                            rhs_of(k),
                            start=(k == 0),
                            stop=sstop,
                        )
                ot = hsb.tile([P, 2 * NB], f16, tag=tag + "s")
                if zero_bias:
                    nc.scalar.activation(ot, pp, AF.Tanh)
                else:
                    for m in range(2):
                        nc.scalar.activation(
                            ot[:, m * NB : (m + 1) * NB],
                            pp[:, m * NB : (m + 1) * NB],
                            AF.Tanh,
                            bias=bias[:, m : m + 1],
                        )
                return ot

            htb = layer(
                "h",
                lambda m: [(w1t[:, m * P : (m + 1) * P], True)],
                lambda k: xt[0:NF, :],
                b1t,
            )
            hts = [htb[:, 0:NB], htb[:, NB : 2 * NB]]
            ftb = layer(
                "f",
                lambda m: [
                    (w2t[:, 0, m * P : (m + 1) * P], False),
                    (w2t[:, 1, m * P : (m + 1) * P], True),
                ],
                lambda k: hts[k],
                b2t,
            )
            fts = [ftb[:, 0:NB], ftb[:, NB : 2 * NB]]
            ctb = layer(
                "c",
                lambda m: [
                    (w3t[:, 0, m * P : (m + 1) * P], False),
                    (w3t[:, 1, m * P : (m + 1) * P], True),
                ],
                lambda k: fts[k],
                b3t,
            )
            cts = [ctb[:, 0:NB], ctb[:, NB : 2 * NB]]

            # L4 batch-major per j-block: T_soft[128,20] = (c^T block).T @ w4.
            # All 4 j-blocks share one PSUM tile (one bank) so the soft/pred
            # staging below is 2 batched DVE ops per tile.
            sp = spp.tile([P, 4 * FORE], f32, tag="sp")
            for jl in range(4):
                for k in range(2):
                    nc.tensor.matmul(
                        sp[:, jl * FORE : (jl + 1) * FORE],
                        cts[k][:, jl * P : (jl + 1) * P],
                        w4t[:, k, :],
                        start=(k == 0),
                        stop=(k == 1),
                    )
            sp3 = sp.rearrange("p (q c) -> p q c", c=FORE)
            b4b = b4t.unsqueeze(1).broadcast_to((P, 4, FORE))
            r, qloc = st3_region_of(4 * t)
            soft = st3ds[r][:, qloc : qloc + 4, 0:FORE]
            pred = st3ds[r][:, qloc : qloc + 4, FORE : 2 * FORE]
            phys = st3ds[r][:, qloc : qloc + 4, 2 * FORE : 3 * FORE]
            nc.vector.tensor_tensor(out=soft, in0=sp3, in1=b4b, op=ALU.add)
            nc.vector.scalar_tensor_tensor(
                out=pred, in0=soft, scalar=lam, in1=phys, op0=ALU.mult, op1=ALU.add
            )

            # chunked output DMAs (each region first-on-its-queue)
            for r2, (t0r, t1r) in enumerate(out_regions):
                if t + 1 == t1r:
                    nc.sync.dma_start(
                        out=out3[:, 4 * t0r : 4 * t1r, :], in_=st3s[r2]
                    )

    _prune_redundant_waits(nc)
    _split_fat_drains(nc)
    return nc


def _split_fat_drains(nc):
    """Split multi-wait drains into chains of single-wait drains.

    Every instruction struct in this walrus build accepts one sync wait;
    the Tile kernel-tail drain gathers all procs on one instruction. A
    sequence of drains on the same in-order queue is semantically
    identical.
    """
    import concourse.mybir as mybir

    fn = nc.m.functions[0]
    for bb in fn.blocks:
        il = bb.instructions
        idx = 0
        while idx < len(il):
            inst = il[idx]
            si = inst.sync_info
            if (
                isinstance(inst, mybir.InstDrain)
                and si
                and si.on_wait
                and len(si.on_wait) > 1
            ):
                waits = list(si.on_wait)
                for j, wt in enumerate(waits[:-1]):
                    d = mybir.InstDrain(name=f"{inst.name}-w{j}", ins=[], outs=[])
                    d.engine = inst.engine
                    d.sync_info = mybir.SyncInfo(on_wait=[wt], on_update=[])
                    try:
                        nc.register_instruction(d, overwrite=True)
                    except Exception:
                        pass
                    il.insert(idx, d)
                    idx += 1
                si.on_wait = [waits[-1]]
            idx += 1


def _prune_redundant_waits(nc):
    """Drop statically-redundant same-proc semaphore waits.

    Tile's slot-rotation deps stamp the released tile's full accessor clock
    onto the next user, including waits on the instruction's *own* in-order
    proc (engine completion sems / its own DMA queue's sem). Those are
    satisfied by program order, but this walrus build only allows ONE sync
    wait per instruction, so the redundant ones must go. A wait is pruned
    only when every increment of its semaphore comes from earlier
    instructions of the same proc stream (verified by cumulative count).
    CoreSim (race detector + deadlock check) validates the pruned program.
    """
    # Same-engine waits are needed only for same-engine RAW hazards (a read
    # racing an earlier posted write from the same engine). In this program:
    #   * PE reads only SBUF and writes only PSUM  -> no PE-self RAW ever
    #   * ACT reads only PSUM/bias and writes SBUF tiles nothing on ACT
    #     reads back                               -> no ACT-self RAW ever
    #   * DVE reads its own writes constantly (physics recurrence, pred
    #     reading soft), EXCEPT the px->xt copies whose only input is
    #     PE-written PSUM                          -> prune only on xt copies
    # WAW/WAR same-engine edges are enforced by in-order execution and the
    # engine's FIFO write path. DMA queue-self waits order transfers on the
    # same FIFO ring, which processes descriptors serially anyway.
    eng_sem_prefix = {
        "EngineType.PE": "PE_",
        "EngineType.DVE": "DVE_",
        "EngineType.Activation": "Activation_",
        "EngineType.SP": "SP_",
        "EngineType.Pool": "Pool_",
    }
    fn = nc.m.functions[0]
    insts = [i for bb in fn.blocks for i in bb.instructions]
    updaters = {}
    for inst in insts:
        si = inst.sync_info
        if si and si.on_update:
            for u in si.on_update:
                nm = getattr(u, "ant_name", None)
                if nm:
                    updaters.setdefault(nm, set()).add(str(inst.engine))
    cum = {}
    pruned = 0
    for inst in insts:
        si = inst.sync_info
        eng = str(inst.engine)
        tname = type(inst).__name__
        try:
            out_ref = inst.outs[0].memref
        except Exception:
            out_ref = ""
        if si and si.on_wait:
            own_updates = set()
            for u in si.on_update or []:
                nm = getattr(u, "ant_name", None)
                if nm:
                    own_updates.add(nm)
            keep = []
            for wt in si.on_wait:
                nm = wt.ant_name
                prunable = False
                if nm and nm.startswith(eng_sem_prefix.get(eng, "\x00")) and (
                    updaters.get(nm, set()) <= {eng}
                ):
                    if eng == "EngineType.PE":
                        prunable = True  # PE never reads PE-written data
                    elif eng == "EngineType.Activation":
                        prunable = True  # ACT never reads ACT-written data
                    elif eng == "EngineType.DVE" and out_ref.startswith("xt_"):
                        prunable = True  # xt copy reads only PE-written PSUM
                if nm and nm.startswith("DMAHW") and tname == "InstDMACopy":
                    # Ring-FIFO ordering wait vs an earlier DMA that shares
                    # the ring. Every DMA pair in this program touches
                    # disjoint memory (chunked inputs / region outputs), and
                    # a ring processes its descriptors serially anyway, so
                    # the wait carries no data hazard.
                    prunable = True
                if prunable and wt.wait_value <= cum.get(nm, 0):
                    pruned += 1
                    continue
                keep.append(wt)
            if len(keep) != len(si.on_wait):
                si.on_wait = keep
        if si and si.on_update:
            for u in si.on_update:
                nm = getattr(u, "ant_name", None)
                if nm:
                    cum[nm] = cum.get(nm, 0) + getattr(u, "update_value", 1)
    return pruned


def _prep_weights(enc_w1, enc_b1, enc_w2, enc_b2, cor_w1, cor_b1, cor_w2, cor_b2):
    f32, f16 = np.float32, np.float16
    WPK = HID + 2 * HID + 2 * HID + 2 * FORE + P
    wpk = np.zeros((P, WPK), f16)
    wpk[:, 5 * HID + 2 * FORE : 5 * HID + 2 * FORE + P] = np.eye(P, dtype=f16)
    wpk[0 : HIST + FORE, 0:HID] = enc_w1.astype(f16)
    wpk[:, HID : 3 * HID] = (
        enc_w2.reshape(2, P, HID).transpose(1, 0, 2).reshape(P, 2 * HID).astype(f16)
    )
    wpk[:, 3 * HID : 5 * HID] = (
        cor_w1.reshape(2, P, HID).transpose(1, 0, 2).reshape(P, 2 * HID).astype(f16)
    )
    wpk[:, 5 * HID : 5 * HID + 2 * FORE] = (
        cor_w2.reshape(2, P, FORE).transpose(1, 0, 2).reshape(P, 2 * FORE).astype(f16)
    )
    BPK = 6 + FORE + P
    bpk = np.zeros((P, BPK), f32)
    bpk[:, 0:2] = enc_b1.reshape(2, P).T
    bpk[:, 2:4] = enc_b2.reshape(2, P).T
    bpk[:, 4:6] = cor_b1.reshape(2, P).T
    bpk[:, 6 : 6 + FORE] = np.broadcast_to(cor_b2.reshape(1, FORE), (P, FORE))
    bpk[:, 6 + FORE : 6 + FORE + P] = np.eye(P, dtype=f32)
    return dict(wpk=wpk, bpk=bpk)


LAST_RESULT = None  # BassKernelResults of the most recent kernel() call


def kernel(history, enc_w1, enc_b1, enc_w2, enc_b2, cor_w1, cor_b1, cor_w2, cor_b2,
           alpha, beta, gamma, tau, lambda_mix):
    from concourse.bass_utils import run_bass_kernel_spmd

    global LAST_RESULT

    history = np.asarray(history, np.float32)
    assert history.shape == (B, HIST)

    def sig(x):
        return float(1.0 / (1.0 + np.exp(-np.float64(x))))

    a = sig(alpha)
    bcoef = sig(beta)
    g = float(abs(np.float64(gamma)))
    lam = sig(lambda_mix)
    c1 = 1.0 - a
    tau_int = int(np.clip(float(tau), 1.0, 18.0))

    zb = not (
        np.any(np.asarray(enc_b1)) or np.any(np.asarray(enc_b2))
        or np.any(np.asarray(cor_b1))
    )
    w = B // NCORES // P  # rows per partition per core
    nc = _build_nc(w, c1, bcoef, g, lam, tau_int, zero_bias=zb)

    shared = _prep_weights(
        np.asarray(enc_w1, np.float32), np.asarray(enc_b1, np.float32),
        np.asarray(enc_w2, np.float32), np.asarray(enc_b2, np.float32),
        np.asarray(cor_w1, np.float32), np.asarray(cor_b1, np.float32),
        np.asarray(cor_w2, np.float32), np.asarray(cor_b2, np.float32),
    )
    rows = B // NCORES
    htail_full = np.ascontiguousarray(history[:, HIST - tau_int :])
    in_maps = [
        {
            "hist": np.ascontiguousarray(history[i * rows : (i + 1) * rows]),
            "htail": htail_full[i * rows : (i + 1) * rows],
            **shared,
        }
        for i in range(NCORES)
    ]

    res = run_bass_kernel_spmd(nc, in_maps, core_ids=list(range(NCORES)))
    LAST_RESULT = res

    preds, physs, softs = [], [], []
    for i in range(NCORES):
        o = np.asarray(res.results[i]["out60"], np.float32).reshape(rows, 60)
        softs.append(o[:, 0:FORE])
        preds.append(o[:, FORE : 2 * FORE])
        physs.append(o[:, 2 * FORE : 3 * FORE])
    T_soft = np.concatenate(softs, 0)
    T_pred = np.concatenate(preds, 0)
    T_physics = np.concatenate(physs, 0)
    return (T_pred, T_physics, T_soft)
